# revision 9
# baseline (speedup 1.0000x reference)
"""Trainium2 Bass kernel for nn_Channel_Attention (channel/XCA attention).

Math restructure (per batch b, x_b = x[b] reshaped [C, N], N = H*W):
    q = Wq^T x, k = Wk^T x, v = Wv^T x (per head h: 24 channels)
    attn_h = softmax_e( (q_h / |q_h|) (k_h / |k_h|)^T * t_h )   [24, 24]
    out = Wp^T (A_bd v) + b  where A_bd = blockdiag(attn_h)
All of q/k only enter through N-dim inner products, so with the channel
Gram matrix XX = x x^T [C, C]:
    R  = XX Wqk                       [C, 2C]
    sq = colsum(Wqk * R)              [2C]   (the |q|^2 / |k|^2 norms)
    G  = Wq^T XX Wk (per-head blocks of Wqk^T R)
and the output collapses to a single folded matmul:
    y = Wfin^T x + b,  Wfin = Wv (A_bd^T Wp)   [C, C]
This removes the [B,N,3C] qkv tensor entirely: the only O(N) work is the
Gram accumulation and the final matmul (plus an xbar-DMA transpose of x
to token-major for the Gram contraction).

Sharding: data-parallel over batch, 2 batches per core on 8 cores.
"""

import numpy as np

C = 192
HEADS = 8
DHEAD = C // HEADS  # 24
EPS = 1e-12
NEG_BIG = -1e30

_nc_cache = {}


def _build_nc(n_batch, n_tok, repeats=1):
    from contextlib import ExitStack

    import concourse.bass as bass
    import concourse.mybir as mybir
    import concourse.tile as tile
    from concourse import bacc

    f32 = mybir.dt.float32
    bf16 = mybir.dt.bfloat16
    MUL = mybir.AluOpType.mult
    ADD = mybir.AluOpType.add

    nc = bacc.Bacc(None, target_bir_lowering=False)
    x = nc.dram_tensor("x", [n_batch, C, n_tok], f32, kind="ExternalInput")
    w_qkv = nc.dram_tensor("w_qkv", [C, 3 * C], f32, kind="ExternalInput")
    w_proj = nc.dram_tensor("w_proj", [C, C], f32, kind="ExternalInput")
    b_proj = nc.dram_tensor("b_proj", [C, 1], f32, kind="ExternalInput")
    temperature = nc.dram_tensor("temperature", [1, HEADS], f32, kind="ExternalInput")
    y = nc.dram_tensor("y", [n_batch, C, n_tok], f32, kind="ExternalOutput")

    NCH = n_tok // 512       # 512-token chunks
    NSUB = n_tok // 128      # 128-token subchunks (xbar transpose + Gram)

    with tile.TileContext(nc) as tc, ExitStack() as ctx:
        consts = ctx.enter_context(tc.tile_pool(name="consts", bufs=1))
        xf_pool = ctx.enter_context(tc.tile_pool(name="xf", bufs=4))
        xbf_pool = ctx.enter_context(tc.tile_pool(name="xbf", bufs=2))
        xt_pool = ctx.enter_context(tc.tile_pool(name="xt", bufs=6))
        sb_small = ctx.enter_context(tc.tile_pool(name="sbs", bufs=2))
        yout_pool = ctx.enter_context(tc.tile_pool(name="yout", bufs=4))
        xx_psum = ctx.enter_context(tc.tile_pool(name="xxp", bufs=1, space="PSUM"))
        sp_psum = ctx.enter_context(tc.tile_pool(name="spp", bufs=2, space="PSUM"))
        y_psum = ctx.enter_context(tc.tile_pool(name="yp", bufs=2, space="PSUM"))

        # ---- constants / weights ----
        Wqk0 = consts.tile([128, 2 * C], f32, tag="wqk0")
        nc.sync.dma_start(out=Wqk0, in_=w_qkv[0:128, 0 : 2 * C])
        Wqk1 = consts.tile([64, 2 * C], f32, tag="wqk1")
        nc.sync.dma_start(out=Wqk1, in_=w_qkv[128:C, 0 : 2 * C])
        Wp0 = consts.tile([96, C], f32, tag="wp0")
        nc.sync.dma_start(out=Wp0, in_=w_proj[0:96, :])
        Wp1 = consts.tile([96, C], f32, tag="wp1")
        nc.sync.dma_start(out=Wp1, in_=w_proj[96:C, :])
        Wv0 = consts.tile([96, C], f32, tag="wv0")
        nc.sync.dma_start(out=Wv0, in_=w_qkv[0:96, 2 * C : 3 * C])
        Wv1 = consts.tile([96, C], f32, tag="wv1")
        nc.sync.dma_start(out=Wv1, in_=w_qkv[96:C, 2 * C : 3 * C])
        b0 = consts.tile([128, 1], f32, tag="b0")
        nc.sync.dma_start(out=b0, in_=b_proj[0:128, :])
        b1 = consts.tile([64, 1], f32, tag="b1")
        nc.sync.dma_start(out=b1, in_=b_proj[128:C, :])

        # WvT[j', c] = Wv[c, j'] via DVE 32x32 block transposes
        WvT = [consts.tile([96, C], f32, tag=f"wvt{m}", name=f"wvt{m}") for m in range(2)]
        Wv_t = [Wv0, Wv1]
        for ci in range(6):
            for ji in range(6):
                src = Wv_t[ci // 3][(ci % 3) * 32 : (ci % 3 + 1) * 32,
                                    ji * 32 : (ji + 1) * 32]
                dst = WvT[ji // 3][(ji % 3) * 32 : (ji % 3 + 1) * 32,
                                   ci * 32 : (ci + 1) * 32]
                nc.vector.transpose(dst, src)

        # 1/temperature, broadcast to the q-channel row [1, 2C] (k half = 1.0)
        temp_sb = consts.tile([1, HEADS], f32, tag="tmp")
        nc.sync.dma_start(out=temp_sb, in_=temperature[:, :])
        tinv_full = consts.tile([1, 2 * C], f32, tag="tinv")
        nc.vector.memset(tinv_full, 1.0)
        tinv8 = consts.tile([1, HEADS], f32, tag="tinv8")
        nc.vector.reciprocal(tinv8, temp_sb)
        a = tinv8[:, :]
        brd = bass.AP(tensor=a.tensor, offset=a.offset,
                      ap=[a.ap[0], a.ap[1], [0, DHEAD]])
        nc.vector.tensor_copy(
            tinv_full[:, 0:C].rearrange("p (h e) -> p h e", e=DHEAD), brd)

        # block-diagonal softmax mask (0 on the 4 diag 24x24 blocks, -1e30 off).
        # Compute engines need 32-aligned partition offsets; DMA does not, so
        # the 24-aligned diagonal blocks are zeroed via SBUF->SBUF DMA.
        mask = consts.tile([96, 96], f32, tag="mask")
        nc.vector.memset(mask, NEG_BIG)
        zero24 = consts.tile([24, 24], f32, tag="zero24")
        nc.vector.memset(zero24, 0.0)
        for i in range(4):
            nc.sync.dma_start(
                out=mask[24 * i : 24 * (i + 1), 24 * i : 24 * (i + 1)], in_=zero24)

        ones0 = consts.tile([128, 1], f32, tag="ones0")
        nc.vector.memset(ones0, 1.0)
        ones1 = consts.tile([64, 1], f32, tag="ones1")
        nc.vector.memset(ones1, 1.0)
        one11 = consts.tile([1, 1], f32, tag="one11")
        nc.vector.memset(one11, 1.0)
        ones_row = consts.tile([1, 96], f32, tag="onesr")
        nc.vector.memset(ones_row, 1.0)

        for b in [bb for _ in range(repeats) for bb in range(n_batch)]:
            # ================= stage A: load, cast, transpose, Gram =========
            # packed bf16 copy of x[b]: cols [0,n_tok) = channels 0:128.
            # cols [n_tok, 1.5 n_tok): channels 128:192, first token half on
            # partitions 0:64, second half on partitions 64:128 (avoids
            # wasting a 64-partition-wide stripe of SBUF).
            nhalf = n_tok // 2
            x_bf = xbf_pool.tile([128, n_tok + nhalf], bf16, tag="xbf")
            x_bf0 = x_bf[0:128, 0:n_tok]

            def xbf1_slice(tok0, w):
                po = 64 * (tok0 // nhalf)
                local = n_tok + tok0 % nhalf
                return x_bf[po : po + 64, local : local + w]
            XXp0 = xx_psum.tile([128, 192], f32, tag="xx0")
            XXp1 = xx_psum.tile([64, 192], f32, tag="xx1")
            XX0 = XXp0[0:128, 0:192]  # XX[c 0:128, c' 0:192]
            XX1 = XXp1[0:64, 0:192]   # XX[c 128:192, c' 0:192]
            for ch in range(NCH):
                sl = slice(ch * 512, (ch + 1) * 512)
                xf0 = xf_pool.tile([128, 512], f32, tag="xf0")
                nc.sync.dma_start(out=xf0, in_=x[b, 0:128, sl])
                xf1 = xf_pool.tile([64, 512], f32, tag="xf1")
                nc.sync.dma_start(out=xf1, in_=x[b, 128:C, sl])
                nc.vector.tensor_copy(x_bf0[:, sl], xf0)
                nc.scalar.copy(xbf1_slice(ch * 512, 512), xf1)
                for s in range(4):
                    i = ch * 4 + s
                    ssl = slice(i * 128, (i + 1) * 128)
                    xt = xt_pool.tile([128, C], bf16, tag="xt")
                    nc.scalar.dma_start_transpose(xt[:, 0:128], x_bf0[:, ssl])
                    nc.scalar.dma_start_transpose(xt[:, 128:C], xbf1_slice(i * 128, 128))
                    nc.tensor.matmul(XX0, lhsT=xt[:, 0:128], rhs=xt[:, 0:C],
                                     start=(i == 0), stop=(i == NSUB - 1))
                    nc.tensor.matmul(XX1, lhsT=xt[:, 128:C], rhs=xt[:, 0:C],
                                     start=(i == 0), stop=(i == NSUB - 1))

            # ================= stage B: tiny algebra =========================
            XXs0 = sb_small.tile([128, C], f32, tag="xxs0")
            nc.scalar.copy(XXs0, XX0)
            XXs1 = sb_small.tile([64, C], f32, tag="xxs1")
            nc.scalar.copy(XXs1, XX1)

            # R = XX @ Wqk   [C, 2C]
            Rp0 = sp_psum.tile([128, 512], f32, tag="sp")
            R0 = Rp0[0:128, 0:384]
            nc.tensor.matmul(R0, lhsT=XXs0[:, 0:128], rhs=Wqk0, start=True, stop=False)
            nc.tensor.matmul(R0, lhsT=XXs1[:, 0:128], rhs=Wqk1, start=False, stop=True)
            Rp1 = sp_psum.tile([128, 512], f32, tag="sp")
            R1 = Rp1[0:64, 0:384]
            nc.tensor.matmul(R1, lhsT=XXs0[:, 128:C], rhs=Wqk0, start=True, stop=False)
            nc.tensor.matmul(R1, lhsT=XXs1[:, 128:C], rhs=Wqk1, start=False, stop=True)
            Rs0 = sb_small.tile([128, 2 * C], f32, tag="rs0")
            nc.scalar.copy(Rs0, R0)
            Rs1 = sb_small.tile([64, 2 * C], f32, tag="rs1")
            nc.vector.tensor_copy(Rs1, R1)

            # sq[j] = sum_c Wqk[c,j] R[c,j]; nrm = max(sqrt(sq), eps) / temp
            WR0 = sb_small.tile([128, 2 * C], f32, tag="wr0")
            nc.vector.tensor_tensor(WR0, Wqk0, Rs0, MUL)
            WR1 = sb_small.tile([64, 2 * C], f32, tag="wr1")
            nc.vector.tensor_tensor(WR1, Wqk1, Rs1, MUL)
            sqp = sp_psum.tile([128, 512], f32, tag="sp")
            sq = sqp[0:1, 0:384]
            nc.tensor.matmul(sq, lhsT=ones0, rhs=WR0, start=True, stop=False)
            nc.tensor.matmul(sq, lhsT=ones1, rhs=WR1, start=False, stop=True)
            sqs = sb_small.tile([1, 2 * C], f32, tag="sqs")
            nc.scalar.sqrt(sqs, sq)
            nrm = sb_small.tile([1, 2 * C], f32, tag="nrm")
            nc.vector.tensor_scalar_max(nrm, sqs, EPS)
            nrm_eff = sb_small.tile([1, 2 * C], f32, tag="nrme")
            nc.vector.tensor_tensor(nrm_eff, nrm, tinv_full, MUL)

            # rq columns [96, 2] = 1 / nrm_eff[q-cols] (PE K=1 transpose trick)
            rqp = sp_psum.tile([128, 512], f32, tag="sp")
            for t in range(2):
                nc.tensor.matmul(rqp[0:96, t : t + 1],
                                 lhsT=nrm_eff[0:1, 96 * t : 96 * (t + 1)],
                                 rhs=one11, start=True, stop=True)
            rqn = sb_small.tile([96, 2], f32, tag="rqn")
            nc.scalar.copy(rqn, rqp[0:96, 0:2])
            rqc = sb_small.tile([96, 2], f32, tag="rqc")
            nc.vector.reciprocal(rqc, rqn)
            # rk broadcast rows [96, 192] = 1 / nrm_eff[k-cols]
            rkp = sp_psum.tile([128, 512], f32, tag="sp")
            nc.tensor.matmul(rkp[0:96, 0:C], lhsT=ones_row,
                             rhs=nrm_eff[0:1, C : 2 * C], start=True, stop=True)
            rkn = sb_small.tile([96, C], f32, tag="rkn")
            nc.scalar.copy(rkn, rkp[0:96, 0:C])
            rkb = sb_small.tile([96, C], f32, tag="rkb")
            nc.vector.reciprocal(rkb, rkn)

            A_t = []
            for t in range(2):
                # G block [96, 96]: rows q-channels, cols k-channels (same heads)
                g = sp_psum.tile([128, 512], f32, tag="sp")
                gv = g[0:96, 0:96]
                nc.tensor.matmul(gv, lhsT=Wqk0[:, 96 * t : 96 * (t + 1)],
                                 rhs=Rs0[:, C + 96 * t : C + 96 * (t + 1)],
                                 start=True, stop=False)
                nc.tensor.matmul(gv, lhsT=Wqk1[:, 96 * t : 96 * (t + 1)],
                                 rhs=Rs1[:, C + 96 * t : C + 96 * (t + 1)],
                                 start=False, stop=True)
                # logits = G * rq * rk + mask; then row softmax
                L = sb_small.tile([96, 96], f32, tag=f"l{t}")
                nc.vector.scalar_tensor_tensor(
                    L, gv, rqc[:, t : t + 1],
                    rkb[:, 96 * t : 96 * (t + 1)], MUL, MUL)
                L2 = sb_small.tile([96, 96], f32, tag=f"l2{t}")
                nc.vector.tensor_tensor(L2, L, mask, ADD)
                negmax = sb_small.tile([96, 1], f32, tag=f"nm{t}")
                nc.vector.tensor_reduce(negmax, L2, mybir.AxisListType.X,
                                        mybir.AluOpType.max, negate=True)
                E = sb_small.tile([96, 96], f32, tag=f"e{t}")
                ssum = sb_small.tile([96, 1], f32, tag=f"ss{t}")
                nc.scalar.activation(E, L2, mybir.ActivationFunctionType.Exp,
                                     bias=negmax[:, 0:1], scale=1.0, accum_out=ssum)
                rs = sb_small.tile([96, 1], f32, tag=f"rsm{t}")
                nc.vector.reciprocal(rs, ssum)
                A = sb_small.tile([96, 96], f32, tag=f"a{t}")
                nc.vector.tensor_scalar_mul(A, E, rs[:, 0:1])
                A_t.append(A)

            # U = A_bd^T Wp ; Wfin = Wv U  -> stored [c, o] as y-matmul lhsT
            Us = []
            for t in range(2):
                up = sp_psum.tile([128, 512], f32, tag="sp")
                uv = up[0:96, 0:C]
                nc.tensor.matmul(uv, lhsT=A_t[t], rhs=(Wp0 if t == 0 else Wp1),
                                 start=True, stop=True)
                us = sb_small.tile([96, C], f32, tag=f"us{t}")
                nc.scalar.copy(us, uv)
                Us.append(us)
            Wfp0 = sp_psum.tile([128, 512], f32, tag="sp")
            Wf0 = Wfp0[0:128, 0:C]
            nc.tensor.matmul(Wf0, lhsT=WvT[0][:, 0:128], rhs=Us[0], start=True, stop=False)
            nc.tensor.matmul(Wf0, lhsT=WvT[1][:, 0:128], rhs=Us[1], start=False, stop=True)
            Wfp1 = sp_psum.tile([128, 512], f32, tag="sp")
            Wf1 = Wfp1[0:64, 0:C]
            nc.tensor.matmul(Wf1, lhsT=WvT[0][:, 128:C], rhs=Us[0], start=True, stop=False)
            nc.tensor.matmul(Wf1, lhsT=WvT[1][:, 128:C], rhs=Us[1], start=False, stop=True)
            Wfb0 = sb_small.tile([128, C], bf16, tag="wfb0")
            nc.vector.tensor_copy(Wfb0, Wf0)
            # duplicate the 64-row K-tile into both partition halves so its
            # base partition can match xbf1_slice's (PE requires equal bases)
            Wfb1 = sb_small.tile([128, C], bf16, tag="wfb1")
            nc.vector.tensor_copy(Wfb1[0:64, :], Wf1)
            nc.vector.tensor_copy(Wfb1[64:128, :], Wf1)

            # ================= stage C: y = Wfin^T x + b =====================
            for ch in range(NCH):
                sl = slice(ch * 512, (ch + 1) * 512)
                rhs1 = xbf1_slice(ch * 512, 512)
                po = 64 * ((ch * 512) // nhalf)
                y0p = y_psum.tile([128, 512], f32, tag="y0")
                nc.tensor.matmul(y0p, lhsT=Wfb0[:, 0:128], rhs=x_bf0[:, sl],
                                 start=True, stop=False)
                nc.tensor.matmul(y0p, lhsT=Wfb1[po : po + 64, 0:128], rhs=rhs1,
                                 start=False, stop=True)
                y1p = y_psum.tile([64, 512], f32, tag="y1")
                nc.tensor.matmul(y1p, lhsT=Wfb0[:, 128:C], rhs=x_bf0[:, sl],
                                 start=True, stop=False)
                nc.tensor.matmul(y1p, lhsT=Wfb1[po : po + 64, 128:C], rhs=rhs1,
                                 start=False, stop=True)
                ysb0 = yout_pool.tile([128, 512], f32, tag="ys0")
                nc.scalar.add(ysb0, y0p, add=b0[:, 0:1])
                ysb1 = yout_pool.tile([64, 512], f32, tag="ys1")
                nc.vector.tensor_scalar_add(ysb1, y1p, b1[:, 0:1])
                nc.sync.dma_start(out=y[b, 0:128, sl], in_=ysb0)
                nc.sync.dma_start(out=y[b, 128:C, sl], in_=ysb1)

    nc.finalize()
    return nc


def _get_nc(n_batch, n_tok, repeats=1):
    key = (n_batch, n_tok, repeats)
    if key not in _nc_cache:
        _nc_cache[key] = _build_nc(n_batch, n_tok, repeats)
    return _nc_cache[key]


def kernel(x, w_qkv, w_proj, b_proj, temperature):
    from concourse.bass_utils import run_bass_kernel_spmd

    B, Cx, H, W = x.shape
    assert Cx == C
    n_tok = H * W
    n_cores = 8
    bpc = B // n_cores
    nc = _get_nc(bpc, n_tok)

    xr = np.ascontiguousarray(np.asarray(x, dtype=np.float32).reshape(B, C, n_tok))
    wqkv = np.ascontiguousarray(np.asarray(w_qkv, dtype=np.float32))
    wproj = np.ascontiguousarray(np.asarray(w_proj, dtype=np.float32))
    bproj = np.ascontiguousarray(np.asarray(b_proj, dtype=np.float32).reshape(C, 1))
    temp = np.ascontiguousarray(np.asarray(temperature, dtype=np.float32).reshape(1, HEADS))

    in_maps = []
    for c in range(n_cores):
        in_maps.append({
            "x": xr[c * bpc : (c + 1) * bpc],
            "w_qkv": wqkv,
            "w_proj": wproj,
            "b_proj": bproj,
            "temperature": temp,
        })
    res = run_bass_kernel_spmd(nc, in_maps, core_ids=list(range(n_cores)))
    out = np.concatenate([r["y"] for r in res.results], axis=0)
    return out.reshape(B, C, H, W).astype(np.float32)


# revision 17
# speedup vs baseline: 8.7530x; 8.7530x over previous
"""Trainium2 Bass kernel for nn_Channel_Attention (channel/XCA attention).

Math restructure (per batch b, x_b = x[b] reshaped [C, N], N = H*W):
    q = Wq^T x, k = Wk^T x, v = Wv^T x (per head h: 24 channels)
    attn_h = softmax_e( (q_h / |q_h|) (k_h / |k_h|)^T * t_h )   [24, 24]
    out = Wp^T (A_bd v) + b  where A_bd = blockdiag(attn_h)
q and k only enter through N-dim inner products, so everything needed
for the attention matrices lives in the small Gram matrix
    B = qk^T qk   [2C, 2C],  qk = x^T Wqk  (token-major, [N, 2C])
        diag(B)  -> the |q|^2 / |k|^2 norms
        B[q-rows, k-cols] head-diagonal blocks -> G = q k^T
and the output collapses to a single folded matmul:
    y = Wfin^T x + b,  Wfin = Wv (A_bd^T Wp)   [C, C]
The [B,N,3C] qkv tensor is never materialized: the only O(N) work is
the qk matmul + B accumulation (both TensorE) and the final matmul.
All tensor-engine streams are bf16 (4 cols/cycle vs 1/4 for fp32).

Sharding: data-parallel over batch, 2 batches per core on 8 cores.
The tiny [C,C]-scale algebra ("stage B") is batched across the per-core
batches into single wide ops to keep its serial latency off the
critical path.
"""

import numpy as np

C = 192
HEADS = 8
DHEAD = C // HEADS  # 24
EPS = 1e-12
NEG_BIG = -1e30

_nc_cache = {}


def _build_nc(n_batch, n_tok, repeats=1, ablate=()):
    from contextlib import ExitStack

    import concourse.bass as bass
    import concourse.mybir as mybir
    import concourse.tile as tile
    from concourse import bacc
    from concourse.masks import make_identity

    f32 = mybir.dt.float32
    bf16 = mybir.dt.bfloat16
    MUL = mybir.AluOpType.mult
    ADD = mybir.AluOpType.add
    SUB = mybir.AluOpType.subtract

    def bcast(ap, n, axis_after=1):
        """Insert a [0, n] broadcast dim into an AP after `axis_after` dims."""
        dims = list(ap.ap)
        dims = dims[:axis_after] + [[0, n]] + dims[axis_after:]
        return bass.AP(tensor=ap.tensor, offset=ap.offset, ap=dims)

    nc = bacc.Bacc(None, target_bir_lowering=False)
    x = nc.dram_tensor("x", [n_batch, C, n_tok], f32, kind="ExternalInput")
    w_qkv = nc.dram_tensor("w_qkv", [C, 3 * C], f32, kind="ExternalInput")
    w_proj = nc.dram_tensor("w_proj", [C, C], f32, kind="ExternalInput")
    b_proj = nc.dram_tensor("b_proj", [C, 1], f32, kind="ExternalInput")
    temperature = nc.dram_tensor("temperature", [1, HEADS], f32, kind="ExternalInput")
    y = nc.dram_tensor("y", [n_batch, C, n_tok], f32, kind="ExternalOutput")

    NCH = n_tok // 512       # 512-token chunks
    NSUB = n_tok // 128      # 128-token subchunks (qk matmul + B accumulation)
    NB = n_batch
    nhalf = n_tok // 2

    with tile.TileContext(nc) as tc, ExitStack() as ctx:
        consts = ctx.enter_context(tc.tile_pool(name="consts", bufs=1))
        xf_pool = ctx.enter_context(tc.tile_pool(name="xf", bufs=6))
        xbf_pool = ctx.enter_context(tc.tile_pool(name="xbf", bufs=2))
        qks_pool = ctx.enter_context(tc.tile_pool(name="qks", bufs=4))
        sb_small = ctx.enter_context(tc.tile_pool(name="sbs", bufs=1))
        yout_pool = ctx.enter_context(tc.tile_pool(name="yout", bufs=4))
        qk_psum = ctx.enter_context(tc.tile_pool(name="qkp", bufs=2, space="PSUM"))
        b_psum = ctx.enter_context(tc.tile_pool(name="bp", bufs=1, space="PSUM"))
        y_psum = ctx.enter_context(tc.tile_pool(name="yp", bufs=2, space="PSUM"))

        # ---- constants / weights ----
        Wqkf0 = consts.tile([128, 2 * C], f32, tag="wqkf0")
        nc.sync.dma_start(out=Wqkf0, in_=w_qkv[0:128, 0 : 2 * C])
        Wqkf1 = consts.tile([64, 2 * C], f32, tag="wqkf1")
        nc.sync.dma_start(out=Wqkf1, in_=w_qkv[128:C, 0 : 2 * C])
        Wqk0 = consts.tile([128, 2 * C], bf16, tag="wqk0")
        nc.vector.tensor_copy(Wqk0, Wqkf0)
        # 64-row K-tile duplicated into both partition halves (the second
        # token half of x_bf lives at base partition 64; PE needs equal
        # lhsT/rhs base partitions)
        Wqk1 = consts.tile([128, 2 * C], bf16, tag="wqk1")
        nc.vector.tensor_copy(Wqk1[0:64, :], Wqkf1)
        nc.vector.tensor_copy(Wqk1[64:128, :], Wqkf1)
        Wp0 = consts.tile([96, C], f32, tag="wp0")
        nc.sync.dma_start(out=Wp0, in_=w_proj[0:96, :])
        Wp1 = consts.tile([96, C], f32, tag="wp1")
        nc.sync.dma_start(out=Wp1, in_=w_proj[96:C, :])
        Wv0 = consts.tile([96, C], f32, tag="wv0")
        nc.sync.dma_start(out=Wv0, in_=w_qkv[0:96, 2 * C : 3 * C])
        Wv1 = consts.tile([96, C], f32, tag="wv1")
        nc.sync.dma_start(out=Wv1, in_=w_qkv[96:C, 2 * C : 3 * C])
        b0 = consts.tile([128, 1], f32, tag="b0")
        nc.sync.dma_start(out=b0, in_=b_proj[0:128, :])
        b1 = consts.tile([64, 1], f32, tag="b1")
        nc.sync.dma_start(out=b1, in_=b_proj[128:C, :])

        # WvT[j', c] = Wv[c, j'] via DVE 32x32 block transposes
        WvT = [consts.tile([96, C], f32, tag=f"wvt{m}", name=f"wvt{m}")
               for m in range(2)]
        Wv_t = [Wv0, Wv1]
        for ci in range(6):
            for ji in range(6):
                src = Wv_t[ci // 3][(ci % 3) * 32 : (ci % 3 + 1) * 32,
                                    ji * 32 : (ji + 1) * 32]
                dst = WvT[ji // 3][(ji % 3) * 32 : (ji % 3 + 1) * 32,
                                   ci * 32 : (ci + 1) * 32]
                nc.vector.transpose(dst, src)

        # 1/temperature broadcast over q-channel cols [1, 2C] (k half = 1.0)
        temp_sb = consts.tile([1, HEADS], f32, tag="tmp")
        nc.sync.dma_start(out=temp_sb, in_=temperature[:, :])
        tinv_full = consts.tile([1, 2 * C], f32, tag="tinv")
        nc.vector.memset(tinv_full, 1.0)
        tinv8 = consts.tile([1, HEADS], f32, tag="tinv8")
        nc.vector.reciprocal(tinv8, temp_sb)
        nc.vector.tensor_copy(
            tinv_full[:, 0:C].rearrange("p (h e) -> p h e", e=DHEAD),
            bcast(tinv8[:, :], DHEAD, axis_after=2))

        # block-diagonal softmax mask (0 on the 4 diag 24x24 blocks, -1e30 off)
        mask = consts.tile([96, 96], f32, tag="mask")
        nc.vector.memset(mask, NEG_BIG)
        zero24 = consts.tile([24, 24], f32, tag="zero24")
        nc.vector.memset(zero24, 0.0)
        for i in range(4):
            nc.sync.dma_start(
                out=mask[24 * i : 24 * (i + 1), 24 * i : 24 * (i + 1)], in_=zero24)

        one11 = consts.tile([1, 1], f32, tag="one11")
        nc.vector.memset(one11, 1.0)
        ones_row = consts.tile([1, 96], f32, tag="onesr")
        nc.vector.memset(ones_row, 1.0)
        ident = consts.tile([128, 128], f32, tag="ident")
        make_identity(nc, ident)

        for rep in range(repeats):
            x_bfs = []
            Gs_t = [sb_small.tile([96, NB, 96], f32, tag=f"gs{t}", name=f"gs{t}")
                    for t in range(2)]
            sqcol = sb_small.tile([128, NB * 3], f32, tag="sqcol", name="sqcol")
            # ====== stage A per batch: load, cast, qk matmul, B += qk^T qk ==
            for b in range(NB):
                # packed bf16 x[b]: cols [0,n_tok) = channels 0:128; cols
                # [n_tok,1.5n_tok) = channels 128:192 with the two token
                # halves stacked on partitions 0:64 / 64:128.
                x_bf = xbf_pool.tile([128, n_tok + nhalf], bf16, tag="xbf",
                                     name="x_bf")
                x_bf0 = x_bf[0:128, 0:n_tok]

                def xbf1_slice(tok0, w, x_bf=x_bf):
                    po = 64 * (tok0 // nhalf)
                    local = n_tok + tok0 % nhalf
                    return x_bf[po : po + 64, local : local + w]

                x_bfs.append((x_bf, x_bf0, xbf1_slice))

                Bp = [b_psum.tile([128, 2 * C], f32, tag=f"b{t}", name=f"Bp{t}")
                      for t in range(3)]
                for ch in range(NCH):
                    sl = slice(ch * 512, (ch + 1) * 512)
                    xf0 = xf_pool.tile([128, 512], f32, tag="xf0", name="xf0")
                    nc.sync.dma_start(out=xf0, in_=x[b, 0:128, sl])
                    xf1 = xf_pool.tile([64, 512], f32, tag="xf1", name="xf1")
                    nc.sync.dma_start(out=xf1, in_=x[b, 128:C, sl])
                    if "cast" not in ablate:
                        nc.vector.tensor_copy(x_bf0[:, sl], xf0)
                        nc.scalar.copy(xbf1_slice(ch * 512, 512), xf1)
                    elif ch == 0:
                        nc.vector.memset(x_bf, 0.01)
                    for s in range(4):
                        i = ch * 4 + s
                        ssl = slice(i * 128, (i + 1) * 128)
                        po = 64 * ((i * 128) // nhalf)
                        # qk[i] = x_chunk^T Wqk   [128 tokens, 2C] (token-major)
                        qkp = qk_psum.tile([128, 2 * C], f32, tag="qk",
                                           name="qkp")
                        nc.tensor.matmul(qkp, lhsT=x_bf0[:, ssl], rhs=Wqk0,
                                         start=True, stop=False)
                        nc.tensor.matmul(qkp, lhsT=xbf1_slice(i * 128, 128),
                                         rhs=Wqk1[po : po + 64, :],
                                         start=False, stop=True)
                        qks = qks_pool.tile([128, 2 * C], bf16, tag="qks",
                                            name="qks")
                        if i % 2 == 0:
                            nc.vector.tensor_copy(qks, qkp)
                        else:
                            nc.scalar.copy(qks, qkp)
                        # B += qk^T qk (3 row-tiles of 128)
                        if "gram" not in ablate or i == 0:
                            one = ("gram" in ablate)
                            for t in range(3):
                                nc.tensor.matmul(
                                    Bp[t], lhsT=qks[:, 128 * t : 128 * (t + 1)],
                                    rhs=qks, start=(i == 0),
                                    stop=(i == NSUB - 1) or one)
                # ---- extract G blocks and diag(B) from PSUM ----
                nc.scalar.copy(Gs_t[0][:, b, :], Bp[0][0:96, C : C + 96])
                nc.scalar.copy(Gs_t[1][0:32, b, :], Bp[0][96:128, C + 96 : 2 * C])
                nc.scalar.copy(Gs_t[1][32:64, b, :], Bp[1][0:32, C + 96 : 2 * C])
                nc.scalar.copy(Gs_t[1][64:96, b, :], Bp[1][32:64, C + 96 : 2 * C])
                for t in range(3):
                    dtmp = sb_small.tile([128, 128], f32, tag="dtmp",
                                         name="dtmp", bufs=2)
                    nc.vector.tensor_tensor(
                        dtmp, Bp[t][:, 128 * t : 128 * (t + 1)], ident, MUL)
                    nc.vector.tensor_reduce(
                        sqcol[:, 3 * b + t : 3 * b + t + 1], dtmp,
                        mybir.AxisListType.X, mybir.AluOpType.add)

            # ============ stage B: tiny algebra, batched over NB ============
            Wfbs = []
            if "stageb" in ablate:
                for b in range(NB):
                    Wfb0 = sb_small.tile([128, C], bf16, tag="wfb0",
                                         name="Wfb0", bufs=2)
                    nc.vector.memset(Wfb0, 0.01)
                    Wfb1 = sb_small.tile([128, C], bf16, tag="wfb1",
                                         name="Wfb1", bufs=2)
                    nc.vector.memset(Wfb1, 0.01)
                    Wfbs.append((Wfb0, Wfb1))
            else:
                # sq as a row [1, NB, 2C]: transpose sqcol via PE, flatten via
                # an SBUF->SBUF DMA ([NB*3, 128] -> [1, NB*384])
                sqtp = y_psum.tile([128, 512], f32, tag="y0", name="sqtp")
                nc.tensor.matmul(sqtp[0 : NB * 3, 0:128], lhsT=sqcol,
                                 rhs=ident, start=True, stop=True)
                sqT = sb_small.tile([NB * 3, 128], f32, tag="sqT", name="sqT")
                nc.scalar.copy(sqT, sqtp[0 : NB * 3, 0:128])
                sqrow = sb_small.tile([1, NB, 2 * C], f32, tag="sqrow",
                                      name="sqrow")
                nc.sync.dma_start(
                    out=sqrow.rearrange("p b j -> p (b j)"), in_=sqT[:, :])
                # nrm = max(sqrt(sq), eps) / temp   [1, NB, 2C]
                sqs = sb_small.tile([1, NB, 2 * C], f32, tag="sqs", name="sqs")
                nc.scalar.sqrt(sqs.rearrange("p b j -> p (b j)"),
                               sqrow.rearrange("p b j -> p (b j)"))
                nrm = sb_small.tile([1, NB, 2 * C], f32, tag="nrm", name="nrm")
                nc.vector.tensor_scalar_max(nrm, sqs, EPS)
                nrm_eff = sb_small.tile([1, NB, 2 * C], f32, tag="nrme",
                                        name="nrm_eff")
                nc.vector.tensor_tensor(nrm_eff, nrm, bcast(tinv_full[:, :], NB),
                                        MUL)

                # rq columns [96, NB*2] (PE K=1 transpose), then reciprocal
                rqp = y_psum.tile([128, 512], f32, tag="y0", name="rqp")
                for b in range(NB):
                    for t in range(2):
                        nc.tensor.matmul(
                            rqp[0:96, 2 * b + t : 2 * b + t + 1],
                            lhsT=nrm_eff[0:1, b, 96 * t : 96 * (t + 1)],
                            rhs=one11, start=True, stop=True)
                rqn = sb_small.tile([96, NB * 2], f32, tag="rqn", name="rqn")
                nc.scalar.copy(rqn, rqp[0:96, 0 : NB * 2])
                rqc = sb_small.tile([96, NB * 2], f32, tag="rqc", name="rqc")
                nc.vector.reciprocal(rqc, rqn)
                # rk broadcast rows [96, NB, 192], then reciprocal
                rkp = y_psum.tile([128, 512], f32, tag="y0", name="rkp")
                nc.tensor.matmul(
                    rkp[0:96, 0 : NB * C], lhsT=ones_row,
                    rhs=nrm_eff[0:1, :, C : 2 * C], start=True, stop=True)
                rkn = sb_small.tile([96, NB, C], f32, tag="rkn", name="rkn")
                nc.scalar.copy(rkn.rearrange("p b j -> p (b j)"),
                               rkp[0:96, 0 : NB * C])
                rkb = sb_small.tile([96, NB, C], f32, tag="rkb", name="rkb")
                nc.vector.reciprocal(rkb, rkn)

                A_t = []
                for t in range(2):
                    # logits: G * rk * rq + mask, viewed [96, NB, 96]
                    L = sb_small.tile([96, NB, 96], f32, tag=f"l{t}", name="L")
                    nc.vector.tensor_tensor(
                        L, Gs_t[t], rkb[:, :, 96 * t : 96 * (t + 1)], MUL)
                    L2 = sb_small.tile([96, NB, 96], f32, tag=f"l2{t}", name="L2")
                    rq_sl = rqc[:, t : NB * 2 : 2]  # [96, NB] strided
                    nc.vector.tensor_tensor(L2, L, bcast(rq_sl, 96, axis_after=2),
                                            MUL)
                    L3 = sb_small.tile([96, NB, 96], f32, tag=f"l3{t}", name="L3")
                    nc.vector.tensor_tensor(L3, L2, bcast(mask[:, :], NB), ADD)
                    mx = sb_small.tile([96, NB], f32, tag=f"mx{t}", name="mx")
                    nc.vector.tensor_reduce(mx, L3, mybir.AxisListType.X,
                                            mybir.AluOpType.max)
                    L4 = sb_small.tile([96, NB, 96], f32, tag=f"l4{t}", name="L4")
                    nc.vector.tensor_tensor(L4, L3, bcast(mx[:, :], 96,
                                                          axis_after=2), SUB)
                    E = sb_small.tile([96, NB, 96], f32, tag=f"e{t}", name="E")
                    nc.scalar.activation(E.rearrange("p b j -> p (b j)"),
                                         L4.rearrange("p b j -> p (b j)"),
                                         mybir.ActivationFunctionType.Exp)
                    ssum = sb_small.tile([96, NB], f32, tag=f"ss{t}", name="ssum")
                    nc.vector.tensor_reduce(ssum, E, mybir.AxisListType.X,
                                            mybir.AluOpType.add)
                    rsm = sb_small.tile([96, NB], f32, tag=f"rsm{t}", name="rsm")
                    nc.vector.reciprocal(rsm, ssum)
                    A = sb_small.tile([96, NB, 96], f32, tag=f"a{t}", name="A")
                    nc.vector.tensor_tensor(A, E, bcast(rsm[:, :], 96,
                                                        axis_after=2), MUL)
                    A_t.append(A)

                # U = A_bd^T Wp ; Wfin = Wv U -> y-matmul lhsT (bf16)
                for b in range(NB):
                    Us = []
                    for t in range(2):
                        up = y_psum.tile([128, 512], f32, tag="y0", name="up")
                        uv = up[0:96, 0:C]
                        nc.tensor.matmul(uv, lhsT=A_t[t][:, b, :],
                                         rhs=(Wp0 if t == 0 else Wp1),
                                         start=True, stop=True)
                        us = sb_small.tile([96, C], f32, tag=f"us{t}", name="us")
                        nc.scalar.copy(us, uv)
                        Us.append(us)
                    Wfp0 = y_psum.tile([128, 512], f32, tag="y0", name="Wfp0")
                    Wf0 = Wfp0[0:128, 0:C]
                    nc.tensor.matmul(Wf0, lhsT=WvT[0][:, 0:128], rhs=Us[0],
                                     start=True, stop=False)
                    nc.tensor.matmul(Wf0, lhsT=WvT[1][:, 0:128], rhs=Us[1],
                                     start=False, stop=True)
                    Wfp1 = y_psum.tile([128, 512], f32, tag="y0", name="Wfp1")
                    Wf1 = Wfp1[0:64, 0:C]
                    nc.tensor.matmul(Wf1, lhsT=WvT[0][:, 128:C], rhs=Us[0],
                                     start=True, stop=False)
                    nc.tensor.matmul(Wf1, lhsT=WvT[1][:, 128:C], rhs=Us[1],
                                     start=False, stop=True)
                    Wfb0 = sb_small.tile([128, C], bf16, tag="wfb0",
                                         name="Wfb0", bufs=2)
                    nc.vector.tensor_copy(Wfb0, Wf0)
                    # duplicate 64-row K-tile into both partition halves (PE
                    # needs equal base partitions between lhsT and rhs)
                    Wfb1 = sb_small.tile([128, C], bf16, tag="wfb1",
                                         name="Wfb1", bufs=2)
                    nc.vector.tensor_copy(Wfb1[0:64, :], Wf1)
                    nc.vector.tensor_copy(Wfb1[64:128, :], Wf1)
                    Wfbs.append((Wfb0, Wfb1))

            # ============ stage C per batch: y = Wfin^T x + b ===============
            for b in range(NB):
                x_bf, x_bf0, xbf1_slice = x_bfs[b]
                Wfb0, Wfb1 = Wfbs[b]
                for ch in range(NCH):
                    sl = slice(ch * 512, (ch + 1) * 512)
                    rhs1 = xbf1_slice(ch * 512, 512)
                    po = 64 * ((ch * 512) // nhalf)
                    y0p = y_psum.tile([128, 512], f32, tag="y0", name="y0p")
                    nc.tensor.matmul(y0p, lhsT=Wfb0[:, 0:128], rhs=x_bf0[:, sl],
                                     start=True, stop=False)
                    nc.tensor.matmul(y0p, lhsT=Wfb1[po : po + 64, 0:128],
                                     rhs=rhs1, start=False, stop=True)
                    y1p = y_psum.tile([64, 512], f32, tag="y1", name="y1p",
                                      bufs=1)
                    nc.tensor.matmul(y1p, lhsT=Wfb0[:, 128:C], rhs=x_bf0[:, sl],
                                     start=True, stop=False)
                    nc.tensor.matmul(y1p, lhsT=Wfb1[po : po + 64, 128:C],
                                     rhs=rhs1, start=False, stop=True)
                    ysb0 = yout_pool.tile([128, 512], f32, tag="ys0", name="ysb0")
                    nc.scalar.add(ysb0, y0p, add=b0[:, 0:1])
                    ysb1 = yout_pool.tile([64, 512], f32, tag="ys1", name="ysb1")
                    nc.vector.tensor_scalar_add(ysb1, y1p, b1[:, 0:1])
                    nc.sync.dma_start(out=y[b, 0:128, sl], in_=ysb0)
                    nc.sync.dma_start(out=y[b, 128:C, sl], in_=ysb1)

    nc.finalize()
    return nc


def _get_nc(n_batch, n_tok, repeats=1, ablate=()):
    key = (n_batch, n_tok, repeats, tuple(ablate))
    if key not in _nc_cache:
        _nc_cache[key] = _build_nc(n_batch, n_tok, repeats, ablate)
    return _nc_cache[key]


def kernel(x, w_qkv, w_proj, b_proj, temperature):
    from concourse.bass_utils import run_bass_kernel_spmd

    B, Cx, H, W = x.shape
    assert Cx == C
    n_tok = H * W
    n_cores = 8
    bpc = B // n_cores
    nc = _get_nc(bpc, n_tok)

    xr = np.ascontiguousarray(np.asarray(x, dtype=np.float32).reshape(B, C, n_tok))
    wqkv = np.ascontiguousarray(np.asarray(w_qkv, dtype=np.float32))
    wproj = np.ascontiguousarray(np.asarray(w_proj, dtype=np.float32))
    bproj = np.ascontiguousarray(np.asarray(b_proj, dtype=np.float32).reshape(C, 1))
    temp = np.ascontiguousarray(
        np.asarray(temperature, dtype=np.float32).reshape(1, HEADS))

    in_maps = []
    for c in range(n_cores):
        in_maps.append({
            "x": xr[c * bpc : (c + 1) * bpc],
            "w_qkv": wqkv,
            "w_proj": wproj,
            "b_proj": bproj,
            "temperature": temp,
        })
    res = run_bass_kernel_spmd(nc, in_maps, core_ids=list(range(n_cores)))
    out = np.concatenate([r["y"] for r in res.results], axis=0)
    return out.reshape(B, C, H, W).astype(np.float32)
